# revision 1
# baseline (speedup 1.0000x reference)
"""Trainium2 Bass kernel for nn_MHAttentionMap (scrambled-reshape variant).

Math (derived from the reference's permute/reshape semantics):
    ql = q @ Wq^T + bq                  # [A, B, H]
    kl = k @ Wk^T + bk                  # [B, H]
    logits[alpha, m] = fact * sum_a ql[a, alpha, m] * kl[a, m]   # m in [0, H)
    out[alpha, beta, n] = softmax_n(logits[alpha, 8*beta + n])   # groups of 8

Sharding: data-parallel over alpha (q's second axis), 32 columns per core,
no collectives. The dominant GEMM (q @ Wq^T, 550 GFLOP) runs on PE in bf16
with f32 PSUM accumulation; the tiny replicated kl projection (0.4% of the
FLOPs) is folded on the host into the klT weight table.

Toolchain constraint: this walrus build allows only ONE semaphore wait per
matmul/DMA instruction. Therefore (a) all HWDGE DMAs are collapsed onto a
single FIFO semaphore proc, and (b) every PE input is staged through a DVE
copy so matmuls only ever wait on the DVE sem.
"""

import numpy as np

import concourse.bass as bass
import concourse.mybir as mybir
import concourse.tile_sem_assignment as _tsa
from concourse.tile import TileContext
from concourse.bass_utils import run_bass_kernel_spmd

_tsa.NUM_HWDGE_SEMS = 1  # all nc.sync DMAs share one FIFO ring/semaphore

A = 256          # q leading axis (contracted in the output)
B = 256          # q second axis (sharded)
H = 2048         # hidden
NH = 8           # heads (softmax group)
NCORES = 8
J = B // NCORES  # 32 alpha columns per core
FACT = float((H / NH) ** -0.5)

F32 = mybir.dt.float32
BF16 = mybir.dt.bfloat16

HC = H // 128    # 16 contraction chunks
MT = H // 128    # 16 m tiles
AGN = 16         # a-groups (16 a-values x 32 j = 512 free)
AGS = A // AGN   # 16 a per group
NMH = 2          # m-half passes (SBUF: only half of WqT resident)
MTH = MT // NMH  # 8 m-tiles per half

_CACHE = {}


def _build():
    nc = bass.Bass()
    qsT = nc.dram_tensor("qsT", [H, A * J], F32, kind="ExternalInput")
    WqT = nc.dram_tensor("WqT", [H, H], F32, kind="ExternalInput")
    klT = nc.dram_tensor("klT", [128, MT, A], F32, kind="ExternalInput")
    bqk = nc.dram_tensor("bqk", [128, MT], F32, kind="ExternalInput")
    out = nc.dram_tensor("out", [J, H], F32, kind="ExternalOutput")

    ident_d = nc.inline_tensor(np.eye(128, dtype=np.float32), name="ident")
    g_np = np.kron(np.eye(16, dtype=np.float32), np.ones((8, 1), np.float32))
    g_d = nc.inline_tensor(g_np, name="gmat")            # [128, 16]
    gt_d = nc.inline_tensor(np.ascontiguousarray(g_np.T), name="gtmat")  # [16, 128]

    mult = mybir.AluOpType.mult
    add = mybir.AluOpType.add

    with TileContext(nc, linearize=_CACHE.get("linearize", False)) as tc:
        with (
            tc.tile_pool(name="const", bufs=1) as cpool,
            tc.tile_pool(name="stg", bufs=2) as spool,
            tc.tile_pool(name="wq", bufs=2) as wqpool,
            tc.tile_pool(name="qb", bufs=2) as qpool,
            tc.tile_pool(name="acc", bufs=1) as apool,
            tc.tile_pool(name="mpsum", bufs=8, space="PSUM") as mpsum,
        ):
            # ---- constants: DMA to staging, DVE-copy to PE-visible tiles ----
            ident_s = cpool.tile([128, 128], F32, name="ident_s")
            nc.sync.dma_start(ident_s[:], ident_d[:])
            g_s = cpool.tile([128, 16], F32, name="g_s")
            nc.sync.dma_start(g_s[:], g_d[:])
            gt_s = cpool.tile([16, 128], F32, name="gt_s")
            nc.sync.dma_start(gt_s[:], gt_d[:])
            ident_sb = cpool.tile([128, 128], F32, name="ident_sb")
            nc.vector.tensor_copy(ident_sb[:], ident_s[:])
            g_sb = cpool.tile([128, 16], F32, name="g_sb")
            nc.vector.tensor_copy(g_sb[:], g_s[:])
            gt_sb = cpool.tile([16, 128], F32, name="gt_sb")
            nc.vector.tensor_copy(gt_sb[:], gt_s[:])

            klT_sb = cpool.tile([128, MT, A], F32, name="klT_sb")
            nc.sync.dma_start(klT_sb[:], klT[:])
            bqk_sb = cpool.tile([128, MT], F32, name="bqk_sb")
            nc.sync.dma_start(bqk_sb[:], bqk[:])

            # ---- accumulators ----
            s_all = apool.tile([128, MT, J], F32, name="s_all")
            nc.vector.memset(s_all[:], 0.0)

            # ---- main GEMM + weighted reduce ----
            for mh in range(NMH):
                wq_sb = wqpool.tile([128, HC, MTH * 128], BF16, name="wq_sb")
                for wc in range(4):
                    wst = spool.tile([128, HC // 4, MTH * 128], F32,
                                     name="wst", tag="stg")
                    nc.sync.dma_start(
                        wst[:],
                        WqT[:, mh * MTH * 128 : (mh + 1) * MTH * 128]
                        .rearrange("(c p) m -> p c m", p=128)
                        [:, wc * (HC // 4) : (wc + 1) * (HC // 4), :],
                    )
                    nc.vector.tensor_copy(
                        wq_sb[:, wc * (HC // 4) : (wc + 1) * (HC // 4), :], wst[:]
                    )
                for ag in range(AGN):
                    qblk = qpool.tile([128, HC, AGS * J], BF16, name="qblk")
                    for qc in range(2):
                        qst = spool.tile([128, HC // 2, AGS * J], F32,
                                         name="qst", tag="stg")
                        nc.sync.dma_start(
                            qst[:],
                            qsT[:, ag * AGS * J : (ag + 1) * AGS * J]
                            .rearrange("(c p) x -> p c x", p=128)
                            [:, qc * (HC // 2) : (qc + 1) * (HC // 2), :],
                        )
                        nc.vector.tensor_copy(
                            qblk[:, qc * (HC // 2) : (qc + 1) * (HC // 2), :],
                            qst[:],
                        )
                    pss = []
                    for mtl in range(MTH):
                        ps = mpsum.tile([128, AGS * J], F32, name="ps", tag="ps")
                        pss.append(ps)
                        for hc in range(HC):
                            nc.tensor.matmul(
                                ps[:],
                                wq_sb[:, hc, mtl * 128 : (mtl + 1) * 128],
                                qblk[:, hc, :],
                                start=(hc == 0),
                                stop=(hc == HC - 1),
                            )
                    # interleave the weighted reduce across m-tiles so the
                    # per-(m,j) accumulator chains don't serialize the DVE
                    for al in range(AGS):
                        a = ag * AGS + al
                        for mtl in range(MTH):
                            mtg = mh * MTH + mtl
                            nc.vector.scalar_tensor_tensor(
                                out=s_all[:, mtg, :],
                                in0=pss[mtl][:, al * J : (al + 1) * J],
                                scalar=klT_sb[:, mtg, a : a + 1],
                                in1=s_all[:, mtg, :],
                                op0=mult,
                                op1=add,
                            )

            # ---- bq bias fold: s[m, j] += bq[m] * sum_a kl[a, m] (host-made) ----
            for mtg in range(MT):
                nc.vector.tensor_scalar_add(
                    s_all[:, mtg, :], s_all[:, mtg, :], bqk_sb[:, mtg : mtg + 1]
                )

            # ---- softmax over groups of 8 along m (partition dim) ----
            # logits ~ N(0,1): exp without max-subtraction is safe in f32.
            e_all = apool.tile([128, MT, J], F32, name="e_all")
            nc.scalar.activation(
                e_all[:], s_all[:], mybir.ActivationFunctionType.Exp
            )
            # group sums: Z[g, (mt, j)] = sum_{m in g} e[m, mt, j]
            zp = mpsum.tile([16, MT, J], F32, name="zp", tag="ps")
            for mtg in range(MT):
                nc.tensor.matmul(
                    zp[:, mtg, :], g_sb[:], e_all[:, mtg, :], start=True, stop=True
                )
            rz_sb = apool.tile([16, MT, J], F32, name="rz_sb")
            nc.vector.reciprocal(rz_sb[:], zp[:])
            # replicate back: rrep[m, (mt, j)] = rz[m//8, (mt, j)]
            rp = mpsum.tile([128, MT, J], F32, name="rp", tag="ps")
            nc.tensor.matmul(rp[:], gt_sb[:], rz_sb[:], start=True, stop=True)
            w_all = apool.tile([128, MT, J], F32, name="w_all")
            nc.vector.tensor_tensor(w_all[:], e_all[:], rp[:], op=mult)

            # ---- transpose [m, j] -> [j, m] and store ----
            wT = apool.tile([J, MT, 128], F32, name="wT")
            for tpi in range(4):
                tp = mpsum.tile([J, 4, 128], F32, name="tp", tag="ps")
                for k4 in range(4):
                    mtg = tpi * 4 + k4
                    nc.tensor.transpose(
                        tp[:, k4, :], w_all[:, mtg, :], ident_sb[:]
                    )
                nc.vector.tensor_copy(wT[:, tpi * 4 : (tpi + 1) * 4, :], tp[:])
            nc.sync.dma_start(out[:], wT[:])

    _hoist_waits(nc)
    return nc


def _hoist_waits(nc):
    """This walrus build allows only one semaphore wait per TPB/DMA
    instruction. Hoist all-but-one wait of each instruction onto standalone
    EventSemaphore sync ops on the same engine, issued immediately before —
    the engine sequencer executes in order, so semantics are unchanged."""
    skip = ("InstEventSemaphore", "InstCall", "InstISA")
    for f in nc.m.functions:
        for bb in f.blocks:
            out = []
            for inst in bb.instructions:
                si = inst.sync_info
                if (
                    si is not None
                    and si.on_wait
                    and len(si.on_wait) > 1
                    and type(inst).__name__ not in skip
                ):
                    waits = list(si.on_wait)
                    for w in waits[:-1]:
                        es = mybir.InstEventSemaphore(
                            name=f"{inst.name}-w{len(out)}",
                            engine=inst.engine,
                            sync_info=bass_rust.SyncInfo(
                                on_wait=[w], on_update=[]
                            ),
                        )
                        out.append(es)
                    si.on_wait = waits[-1:]
                out.append(inst)
            bb.instructions = out


import bass_rust  # noqa: E402  (SyncInfo for _hoist_waits)


def _get_nc():
    if "nc" not in _CACHE:
        _CACHE["nc"] = _build()
    return _CACHE["nc"]


def kernel(q, k, Wq, bq, Wk, bk):
    q = np.asarray(q, dtype=np.float32)
    k = np.asarray(k, dtype=np.float32)
    Wq = np.asarray(Wq, dtype=np.float32)
    bq = np.asarray(bq, dtype=np.float32)
    Wk = np.asarray(Wk, dtype=np.float32)
    bk = np.asarray(bk, dtype=np.float32)

    WqT = np.ascontiguousarray(Wq.T)
    # tiny replicated projection on host: kl = k @ Wk^T + bk, fact folded in
    kl = (k @ Wk.T + bk) * np.float32(FACT)          # [A, H] == kl[a, m]
    klT = np.ascontiguousarray(
        kl.T.reshape(MT, 128, A).transpose(1, 0, 2)   # [128, mt, a]
    )
    bqk_m = bq * kl.sum(axis=0)                       # [H]
    bqk = np.ascontiguousarray(bqk_m.reshape(MT, 128).T)  # [128, mt]

    in_maps = []
    for i in range(NCORES):
        qsT = np.ascontiguousarray(
            q[:, i * J : (i + 1) * J, :].transpose(2, 0, 1)
        ).reshape(H, A * J)
        in_maps.append({"qsT": qsT, "WqT": WqT, "klT": klT, "bqk": bqk})

    nc = _get_nc()
    _CACHE["last_in_maps"] = in_maps
    res = run_bass_kernel_spmd(nc, in_maps, core_ids=list(range(NCORES)))
    outs = [r["out"].reshape(J, B, NH) for r in res.results]
    return np.concatenate(outs, axis=0).reshape(A, B, NH, 1, 1)



# revision 4
# speedup vs baseline: 3.1116x; 3.1116x over previous
"""Trainium2 Bass kernel for nn_MHAttentionMap (scrambled-reshape variant).

Math (derived from the reference's permute/reshape semantics):
    ql = q @ Wq^T + bq                  # [A, B, H]
    kl = fact * (k @ Wk^T + bk)         # [A, H]  (A == B == 256 here)
    logits[alpha, m] = sum_a ql[a, alpha, m] * kl[a, m]
    out[alpha, beta, n] = softmax_n(logits[alpha, 8*beta + n])

End-to-end wall time is dominated by the ~40 MB/s axon tunnel, so the
kernel is organized to minimize shipped bytes and host-side copies:

  * Shard over `a` (q's LEADING axis, the contraction axis of the
    logits): per-core q slices are zero-copy views, shipped once (not
    replicated) as bf16 -- 256 MB total instead of 512 MB f32.
  * Wq^T is shipped bf16 (replicated, 8 MB/core).  The tiny kl
    projection runs on the host and ships as a 256 KB/core table.
  * q arrives in its natural [x, h] layout; the h-on-partitions
    layout the PE needs is produced on-device with PE transposes
    (~0.2 ms) instead of a ~7 s host-side numpy transpose.
  * Each core returns partial logits (2 MB); the host sums the 8
    partials, folds the bq bias, and applies the groups-of-8 softmax
    (sub-0.1 s of numpy on 0.5 M elements).

Toolchain constraint: this walrus build allows only ONE semaphore wait
per matmul/DMA instruction.  Therefore (a) all HWDGE DMAs are collapsed
onto a single FIFO semaphore proc, and (b) every PE input is staged
through a DVE copy so matmuls only ever wait on the DVE sem.
"""

import numpy as np
import ml_dtypes

import concourse.bass as bass
import concourse.mybir as mybir
import concourse.tile_sem_assignment as _tsa
from concourse.tile import TileContext
from concourse.bass_utils import run_bass_kernel_spmd

_tsa.NUM_HWDGE_SEMS = 1  # all nc.sync DMAs share one FIFO ring/semaphore

A = 256          # q leading axis == contraction axis of the logits (sharded)
B = 256          # q second axis (alpha)
H = 2048         # hidden
NH = 8           # heads (softmax group)
NCORES = 8
AL = A // NCORES # 32 a-values per core
FACT = float((H / NH) ** -0.5)

F32 = mybir.dt.float32
BF16 = mybir.dt.bfloat16

HC = H // 128    # 16 h (contraction) blocks
MT = H // 128    # 16 m tiles
XL = AL * B      # 8192 (a, alpha) rows per core
XC = XL // 512   # 16 x-chunks of 512 rows (= 2 a-values x 256 alpha)

_CACHE = {}


def _build():
    nc = bass.Bass()
    qb = nc.dram_tensor("qb", [XL, H], BF16, kind="ExternalInput")
    WqT = nc.dram_tensor("WqT", [H, H], BF16, kind="ExternalInput")
    klT = nc.dram_tensor("klT", [128, MT, AL], F32, kind="ExternalInput")
    out = nc.dram_tensor("out", [128, MT, B], F32, kind="ExternalOutput")

    ident_d = nc.inline_tensor(
        np.eye(128, dtype=ml_dtypes.bfloat16), name="ident"
    )

    mult = mybir.AluOpType.mult
    add = mybir.AluOpType.add

    with TileContext(nc, linearize=_CACHE.get("linearize", False)) as tc:
        with (
            tc.tile_pool(name="const", bufs=1) as cpool,
            tc.tile_pool(name="stg", bufs=2) as spool,
            tc.tile_pool(name="wq", bufs=1) as wqpool,
            tc.tile_pool(name="qrow", bufs=2) as qrpool,
            tc.tile_pool(name="qt", bufs=2) as qtpool,
            tc.tile_pool(name="acc", bufs=1) as apool,
            tc.tile_pool(name="tpsum", bufs=2, space="PSUM") as tpsum,
            tc.tile_pool(name="mpsum", bufs=4, space="PSUM") as mpsum,
        ):
            # ---- constants: DMA to staging, DVE-copy to PE-visible tiles ----
            ident_s = cpool.tile([128, 128], BF16, name="ident_s")
            nc.sync.dma_start(ident_s[:], ident_d[:])
            ident_sb = cpool.tile([128, 128], BF16, name="ident_sb")
            nc.vector.tensor_copy(ident_sb[:], ident_s[:])

            klT_s = cpool.tile([128, MT, AL], F32, name="klT_s")
            nc.sync.dma_start(klT_s[:], klT[:])
            klT_sb = cpool.tile([128, MT, AL], F32, name="klT_sb")
            nc.vector.tensor_copy(klT_sb[:], klT_s[:])

            # ---- Wq^T resident in SBUF: [h-part, hb, m] ----
            wq_sb = wqpool.tile([128, HC, H], BF16, name="wq_sb")
            for wc in range(4):
                wst = spool.tile([128, HC // 4, H], BF16, name="wst", tag="stg")
                nc.sync.dma_start(
                    wst[:],
                    WqT.rearrange("(c p) m -> p c m", p=128)
                    [:, wc * (HC // 4) : (wc + 1) * (HC // 4), :],
                )
                nc.vector.tensor_copy(
                    wq_sb[:, wc * (HC // 4) : (wc + 1) * (HC // 4), :], wst[:]
                )

            # ---- accumulator: s_acc[m-part, mt, alpha] ----
            s_acc = apool.tile([128, MT, B], F32, name="s_acc")
            nc.vector.memset(s_acc[:], 0.0)

            # ---- main loop over x-chunks (512 rows = 2 a-values) ----
            for xc in range(XC):
                # load 512 q rows: [p, r, h] with x = xc*512 + r*128 + p
                qst = spool.tile([128, 4, H], BF16, name="qst", tag="stg")
                nc.sync.dma_start(
                    qst[:],
                    qb[xc * 512 : (xc + 1) * 512, :]
                    .rearrange("(r p) h -> p r h", p=128),
                )
                qrow = qrpool.tile([128, 4, H], BF16, name="qrow")
                nc.vector.tensor_copy(qrow[:], qst[:])

                # transpose to qt[h-part, hb, x]: PE transpose per (hb, r)
                qt = qtpool.tile([128, HC, 512], BF16, name="qt")
                for hb in range(HC):
                    tp = tpsum.tile([128, 4, 128], BF16, name="tp", tag="tp")
                    for r in range(4):
                        nc.tensor.transpose(
                            tp[:, r, :],
                            qrow[:, r, hb * 128 : (hb + 1) * 128],
                            ident_sb[:],
                        )
                    nc.vector.tensor_copy(qt[:, hb, :], tp[:])

                # ql chunk + weighted reduce into s_acc
                for mt in range(MT):
                    ps = mpsum.tile([128, 512], F32, name="ps", tag="ps")
                    for hb in range(HC):
                        nc.tensor.matmul(
                            ps[:],
                            wq_sb[:, hb, mt * 128 : (mt + 1) * 128],
                            qt[:, hb, :],
                            start=(hb == 0),
                            stop=(hb == HC - 1),
                        )
                    for ar in range(2):
                        a = xc * 2 + ar
                        nc.vector.scalar_tensor_tensor(
                            out=s_acc[:, mt, :],
                            in0=ps[:, ar * B : (ar + 1) * B],
                            scalar=klT_sb[:, mt, a : a + 1],
                            in1=s_acc[:, mt, :],
                            op0=mult,
                            op1=add,
                        )

            nc.sync.dma_start(out[:], s_acc[:])

    _hoist_waits(nc)
    return nc


def _hoist_waits(nc):
    """This walrus build allows only one semaphore wait per TPB/DMA
    instruction. Hoist all-but-one wait of each instruction onto standalone
    EventSemaphore sync ops on the same engine, issued immediately before --
    the engine sequencer executes in order, so semantics are unchanged."""
    skip = ("InstEventSemaphore", "InstCall", "InstISA")
    for f in nc.m.functions:
        for bb in f.blocks:
            out = []
            for inst in bb.instructions:
                si = inst.sync_info
                if (
                    si is not None
                    and si.on_wait
                    and len(si.on_wait) > 1
                    and type(inst).__name__ not in skip
                ):
                    waits = list(si.on_wait)
                    for w in waits[:-1]:
                        es = mybir.InstEventSemaphore(
                            name=f"{inst.name}-w{len(out)}",
                            engine=inst.engine,
                            sync_info=bass_rust.SyncInfo(
                                on_wait=[w], on_update=[]
                            ),
                        )
                        out.append(es)
                    si.on_wait = waits[-1:]
                out.append(inst)
            bb.instructions = out


import bass_rust  # noqa: E402  (SyncInfo for _hoist_waits)


def _get_nc():
    if "nc" not in _CACHE:
        _CACHE["nc"] = _build()
    return _CACHE["nc"]


def _to_bf16(x):
    """f32 -> bf16 by mantissa truncation (one strided copy; ~5x faster
    than ml_dtypes astype on this 1-cpu host; max extra error 2^-8 rel)."""
    v = x.view(np.uint16)[..., 1::2]  # little-endian high halves
    return np.ascontiguousarray(v).view(ml_dtypes.bfloat16)


def kernel(q, k, Wq, bq, Wk, bk):
    q = np.asarray(q, dtype=np.float32)
    k = np.asarray(k, dtype=np.float32)
    Wq = np.asarray(Wq, dtype=np.float32)
    bq = np.asarray(bq, dtype=np.float32)
    Wk = np.asarray(Wk, dtype=np.float32)
    bk = np.asarray(bk, dtype=np.float32)

    nc = _get_nc()

    qb = _to_bf16(q).reshape(A, B * H)          # [a, (alpha, h)]
    WqTb = np.ascontiguousarray(_to_bf16(Wq).T) # [h, m] bf16
    # tiny replicated projection on host, fact folded in
    kl = (k @ Wk.T + bk) * np.float32(FACT)     # [A, H] == kl[a, m]

    in_maps = []
    for c in range(NCORES):
        qc = qb[c * AL : (c + 1) * AL].reshape(XL, H)   # zero-copy view
        sl = np.ascontiguousarray(kl[c * AL : (c + 1) * AL].T)  # [m, a_l]
        klT = np.ascontiguousarray(
            sl.reshape(MT, 128, AL).transpose(1, 0, 2)  # [128, mt, a_l]
        )
        in_maps.append({"qb": qc, "WqT": WqTb, "klT": klT})

    _CACHE["last_in_maps"] = in_maps
    res = run_bass_kernel_spmd(nc, in_maps, core_ids=list(range(NCORES)))

    # host: sum partials, fold bias, grouped softmax
    S = res.results[0]["out"].copy()
    for r in res.results[1:]:
        S += r["out"]                                   # [128, mt, alpha]
    L = np.ascontiguousarray(S.transpose(2, 1, 0)).reshape(B, H)
    L += bq * kl.sum(axis=0)                            # bias fold
    E = np.exp(L.reshape(B, B, NH))                     # logits ~ N(0,1)
    W = E / E.sum(axis=-1, keepdims=True)
    return W.reshape(A, B, NH, 1, 1).astype(np.float32)


_get_nc()  # build the Bass module at import time


# revision 10
# speedup vs baseline: 4.6178x; 1.4841x over previous
"""Trainium2 Bass kernel for nn_MHAttentionMap (scrambled-reshape variant).

Math (derived from the reference's permute/reshape semantics):
    ql = q @ Wq^T + bq                  # [A, B, H]
    kl = fact * (k @ Wk^T + bk)         # [A, H]  (A == B == 256 here)
    logits[alpha, m] = sum_a ql[a, alpha, m] * kl[a, m]
    out[alpha, beta, n] = softmax_n(logits[alpha, 8*beta + n])

End-to-end wall time is dominated by the ~40 MB/s axon tunnel, so the
kernel is organized to minimize shipped bytes and host-side copies:

  * Shard over `a` (q's LEADING axis, the contraction axis of the
    logits): per-core q slices are zero-copy views, shipped once (not
    replicated) as bf16 -- 256 MB total instead of 512 MB f32.
  * Wq^T is shipped bf16 (replicated, 8 MB/core).  The tiny kl
    projection runs on the host and ships as a 256 KB/core table.
  * q arrives in its natural [x, h] layout; the h-on-partitions
    layout the PE needs is produced on-device with PE transposes
    (~0.2 ms) instead of a ~7 s host-side numpy transpose.
  * Each core returns partial logits (2 MB); the host sums the 8
    partials, folds the bq bias, and applies the groups-of-8 softmax
    (sub-0.1 s of numpy on 0.5 M elements).

Toolchain constraint: this walrus build allows only ONE semaphore wait
per matmul/DMA instruction.  Therefore (a) all HWDGE DMAs are collapsed
onto a single FIFO semaphore proc, and (b) every PE input is staged
through a DVE copy so matmuls only ever wait on the DVE sem.
"""

import numpy as np
import ml_dtypes

import concourse.bass as bass
import concourse.mybir as mybir
import concourse.tile_sem_assignment as _tsa
from concourse.tile import TileContext
from concourse.bass_utils import run_bass_kernel_spmd

_tsa.NUM_HWDGE_SEMS = 1  # all nc.sync DMAs share one FIFO ring/semaphore

A = 256          # q leading axis == contraction axis of the logits (sharded)
B = 256          # q second axis (alpha)
H = 2048         # hidden
NH = 8           # heads (softmax group)
NCORES = 8
AL = A // NCORES # 32 a-values per core
FACT = float((H / NH) ** -0.5)

F32 = mybir.dt.float32
BF16 = mybir.dt.bfloat16

HC = H // 128    # 16 h (contraction) blocks
MT = H // 128    # 16 m tiles
XL = AL * B      # 8192 (a, alpha) rows per core
XC = XL // 512   # 16 x-chunks of 512 rows (= 2 a-values x 256 alpha)

_CACHE = {}


def _build():
    nc = bass.Bass()
    qb = nc.dram_tensor("qb", [XL, H], BF16, kind="ExternalInput")
    # each core ships 1/8 of Wq^T (its h-slice); full Wq^T is rebuilt
    # on-device with an AllGather over NeuronLink (fast) instead of
    # shipping 8 replicas through the ~40 MB/s host tunnel.
    WqTs = nc.dram_tensor("WqTs", [H // NCORES, H], BF16, kind="ExternalInput")
    klT = nc.dram_tensor("klT", [128, MT, AL], F32, kind="ExternalInput")
    out = nc.dram_tensor("out", [128, MT, B], BF16, kind="ExternalOutput")

    ident_d = nc.inline_tensor(
        np.eye(128, dtype=ml_dtypes.bfloat16), name="ident"
    )

    mult = mybir.AluOpType.mult
    add = mybir.AluOpType.add

    with TileContext(nc, linearize=_CACHE.get("linearize", False)) as tc:
        with (
            tc.tile_pool(name="const", bufs=1) as cpool,
            tc.tile_pool(name="stg", bufs=2) as spool,
            tc.tile_pool(name="wq", bufs=1) as wqpool,
            tc.tile_pool(name="qrow", bufs=2) as qrpool,
            tc.tile_pool(name="qt", bufs=2) as qtpool,
            tc.tile_pool(name="acc", bufs=1) as apool,
            tc.tile_pool(name="tpsum", bufs=2, space="PSUM") as tpsum,
            tc.tile_pool(name="mpsum", bufs=4, space="PSUM") as mpsum,
            tc.tile_pool(name="dram", bufs=1, space="DRAM") as dpool,
        ):
            # ---- AllGather the Wq^T shards into a full DRAM copy ----
            wq_in_b = dpool.tile([H // NCORES, H], BF16, name="wq_in_b")
            wq_full = dpool.tile([H, H], BF16, name="wq_full")
            nc.gpsimd.dma_start(wq_in_b[:], WqTs[:])
            nc.gpsimd.collective_compute(
                "AllGather",
                mybir.AluOpType.bypass,
                replica_groups=[list(range(NCORES))],
                ins=[wq_in_b.opt()],
                outs=[wq_full.opt()],
            )
            # ---- constants: DMA to staging, DVE-copy to PE-visible tiles ----
            ident_s = cpool.tile([128, 128], BF16, name="ident_s")
            nc.sync.dma_start(ident_s[:], ident_d[:])
            ident_sb = cpool.tile([128, 128], BF16, name="ident_sb")
            nc.vector.tensor_copy(ident_sb[:], ident_s[:])

            klT_s = cpool.tile([128, MT, AL], F32, name="klT_s")
            nc.sync.dma_start(klT_s[:], klT[:])
            klT_sb = cpool.tile([128, MT, AL], F32, name="klT_sb")
            nc.vector.tensor_copy(klT_sb[:], klT_s[:])

            # ---- Wq^T resident in SBUF: [h-part, hb, m] ----
            wq_sb = wqpool.tile([128, HC, H], BF16, name="wq_sb")
            for wc in range(4):
                wst = spool.tile([128, HC // 4, H], BF16, name="wst", tag="stg")
                nc.sync.dma_start(
                    wst[:],
                    wq_full[:].rearrange("(c p) m -> p c m", p=128)
                    [:, wc * (HC // 4) : (wc + 1) * (HC // 4), :],
                )
                nc.vector.tensor_copy(
                    wq_sb[:, wc * (HC // 4) : (wc + 1) * (HC // 4), :], wst[:]
                )

            # ---- accumulator: s_acc[m-part, mt, alpha] ----
            s_acc = apool.tile([128, MT, B], F32, name="s_acc")
            nc.vector.memset(s_acc[:], 0.0)

            # ---- main loop over x-chunks (512 rows = 2 a-values) ----
            for xc in range(XC):
                # load 512 q rows: [p, r, h] with x = xc*512 + r*128 + p
                qst = spool.tile([128, 4, H], BF16, name="qst", tag="stg")
                nc.sync.dma_start(
                    qst[:],
                    qb[xc * 512 : (xc + 1) * 512, :]
                    .rearrange("(r p) h -> p r h", p=128),
                )
                qrow = qrpool.tile([128, 4, H], BF16, name="qrow")
                nc.vector.tensor_copy(qrow[:], qst[:])

                # transpose to qt[h-part, hb, x]: PE transpose per (hb, r)
                qt = qtpool.tile([128, HC, 512], BF16, name="qt")
                for hb in range(HC):
                    tp = tpsum.tile([128, 4, 128], BF16, name="tp", tag="tp")
                    for r in range(4):
                        nc.tensor.transpose(
                            tp[:, r, :],
                            qrow[:, r, hb * 128 : (hb + 1) * 128],
                            ident_sb[:],
                        )
                    nc.vector.tensor_copy(qt[:, hb, :], tp[:])

                # ql chunk + weighted reduce into s_acc
                for mt in range(MT):
                    ps = mpsum.tile([128, 512], F32, name="ps", tag="ps")
                    for hb in range(HC):
                        nc.tensor.matmul(
                            ps[:],
                            wq_sb[:, hb, mt * 128 : (mt + 1) * 128],
                            qt[:, hb, :],
                            start=(hb == 0),
                            stop=(hb == HC - 1),
                        )
                    for ar in range(2):
                        a = xc * 2 + ar
                        nc.vector.scalar_tensor_tensor(
                            out=s_acc[:, mt, :],
                            in0=ps[:, ar * B : (ar + 1) * B],
                            scalar=klT_sb[:, mt, a : a + 1],
                            in1=s_acc[:, mt, :],
                            op0=mult,
                            op1=add,
                        )

            s_out = apool.tile([128, MT, B], BF16, name="s_out")
            nc.vector.tensor_copy(s_out[:], s_acc[:])
            nc.sync.dma_start(out[:], s_out[:])

    _hoist_waits(nc)
    return nc


def _hoist_waits(nc):
    """This walrus build allows only one semaphore wait per TPB/DMA
    instruction. Hoist all-but-one wait of each instruction onto standalone
    EventSemaphore sync ops on the same engine, issued immediately before --
    the engine sequencer executes in order, so semantics are unchanged."""
    skip = ("InstEventSemaphore", "InstCall", "InstISA")
    for f in nc.m.functions:
        for bb in f.blocks:
            out = []
            for inst in bb.instructions:
                si = inst.sync_info
                if (
                    si is not None
                    and si.on_wait
                    and len(si.on_wait) > 1
                    and type(inst).__name__ not in skip
                ):
                    waits = list(si.on_wait)
                    for w in waits[:-1]:
                        es = mybir.InstEventSemaphore(
                            name=f"{inst.name}-w{len(out)}",
                            engine=inst.engine,
                            sync_info=bass_rust.SyncInfo(
                                on_wait=[w], on_update=[]
                            ),
                        )
                        out.append(es)
                    si.on_wait = waits[-1:]
                out.append(inst)
            bb.instructions = out


import bass_rust  # noqa: E402  (SyncInfo for _hoist_waits)


def _get_nc():
    if "nc" not in _CACHE:
        _CACHE["nc"] = _build()
    return _CACHE["nc"]


def _to_bf16(x):
    """f32 -> bf16 by mantissa truncation (one strided copy; ~5x faster
    than ml_dtypes astype on this 1-cpu host; max extra error 2^-8 rel)."""
    v = x.view(np.uint16)[..., 1::2]  # little-endian high halves
    return np.ascontiguousarray(v).view(ml_dtypes.bfloat16)


def kernel(q, k, Wq, bq, Wk, bk):
    q = np.asarray(q, dtype=np.float32)
    k = np.asarray(k, dtype=np.float32)
    Wq = np.asarray(Wq, dtype=np.float32)
    bq = np.asarray(bq, dtype=np.float32)
    Wk = np.asarray(Wk, dtype=np.float32)
    bk = np.asarray(bk, dtype=np.float32)

    nc = _get_nc()

    qb = _to_bf16(q).reshape(A, B * H)          # [a, (alpha, h)]
    WqTb = np.ascontiguousarray(_to_bf16(Wq).T) # [h, m] bf16
    # tiny replicated projection on host, fact folded in
    kl = (k @ Wk.T + bk) * np.float32(FACT)     # [A, H] == kl[a, m]

    HS = H // NCORES
    in_maps = []
    for c in range(NCORES):
        qc = qb[c * AL : (c + 1) * AL].reshape(XL, H)   # zero-copy view
        wc = WqTb[c * HS : (c + 1) * HS]                # zero-copy h-slice
        sl = np.ascontiguousarray(kl[c * AL : (c + 1) * AL].T)  # [m, a_l]
        klT = np.ascontiguousarray(
            sl.reshape(MT, 128, AL).transpose(1, 0, 2)  # [128, mt, a_l]
        )
        in_maps.append({"qb": qc, "WqTs": wc, "klT": klT})

    _CACHE["last_in_maps"] = in_maps
    res = run_bass_kernel_spmd(nc, in_maps, core_ids=list(range(NCORES)))

    # host: sum partials, fold bias, grouped softmax
    S = res.results[0]["out"].astype(np.float32)
    for r in res.results[1:]:
        S += r["out"]                                   # [128, mt, alpha]
    L = np.ascontiguousarray(S.transpose(2, 1, 0)).reshape(B, H)
    L += bq * kl.sum(axis=0)                            # bias fold
    E = np.exp(L.reshape(B, B, NH))                     # logits ~ N(0,1)
    W = E / E.sum(axis=-1, keepdims=True)
    return W.reshape(A, B, NH, 1, 1).astype(np.float32)


_get_nc()  # build the Bass module at import time
